# revision 26
# baseline (speedup 1.0000x reference)
"""Multi-head attention (B=2, N=2048, D=2048, 16 heads) on 8 NeuronCores.

Sharding: tensor-parallel over heads (2 heads/core) for QKV projections and
attention; one AllToAll per (head, batch) re-shards the attention context
from head-split to row-split; the output projection is row-parallel
(512 rows/core) with the full Wo staged in SBUF on every core.

Performance notes (traced on TRN2; ~455-465us vs 479us prior best):
  - all operands bf16 (halves DMA/SBUF/collective traffic; PE streams
    ~1 col/cycle either way; rel-err budget 2e-2 dwarfs bf16 rounding);
    K-bias dropped exactly (softmax is shift-invariant per query row)
  - attention kc loop: 2 score MMs into a merged [128,1024] two-bank PSUM
    tile, ONE exp ACTIVATE for the qc pair (halves the per-instruction
    PSUM-access bubble that made ACT the phase-2 bottleneck), and the PV
    pair software-pipelined two steps behind in a FLAT stream that spans
    qp/quarter boundaries, so the PE never drains waiting on exp
  - softmax denominators accumulate as merged [128,1024] pt running sums
    on the DVE; the colsum ones-matmuls + reciprocal + DRAM-bounce
    broadcast are deferred into the next qp's kc loop (kc2/kc8) so they
    never stall the PE; ctx is normalized straight from PSUM (ctx double-
    buffered 4 banks + st 4 banks), except the FINAL qp which evicts ctx
    to SBUF immediately so phase 3 can take the banks ~2us after the last
    attention matmul instead of waiting out the bounce round-trip
  - queue discipline matters because the scheduler places A2A-gated DMAs
    right after a queue's last ready work, where they block the head:
    scalar = exp stream + b1 ctxl loads (idle after the last exp);
    gpsimd = bounce + shard writes + collective triggers; sync = Wo
    staging + b0 ctxl loads + phase-3 out tiles; final bounce on sync
  - phase 3 runs two passes per batch (hh=0 sweep over all 8 banks, then
    hh=1 + evict) so a late second-head A2A can never stall a bank
    mid-accumulation; evictions overlap the MM stream
  - warmup: ones tile comes from a memset (no DMA wait) and 16 N=256
    matmuls span the ~3.4us HAM window right before the first real work;
    the 3MB of QKV weights (all consumed inside rc0) round-robin across
    the three DMA queues in 0.25MB slices, and 24 x-tiles pre-issue so
    the rings are deep before the PE catches up

Layout strategy (everything contracts on the SBUF partition axis):
  - host feeds xT = x.T so projections need no on-device transposes
  - Q, K are produced transposed ([head_dim, rows]); V in natural layout by
    using the x-tile as the stationary operand
  - scores are computed transposed: S.T[k_row, q_row] = (K.T)^T . Q.T chunks
  - softmax skips the max-subtraction (scores ~ N(0,1); exp is safe)
  - ctx.T = v^T . P.T accumulates over k_row chunks -> ctx arrives transposed,
    which is exactly what the output projection needs
  - v-bias and o-bias commute out of the kernel: attention rows sum to 1, so
    out = attn@(v0+bv)@Wo.T + bo = device_out + (Wo@bv + bo); host adds it.
"""

import numpy as np
import ml_dtypes

import concourse.bacc as bacc
import concourse.mybir as mybir
import concourse.tile as tile
from concourse.bass_utils import run_bass_kernel_spmd

P = 128          # partitions
B = 2            # batch
SEQ = 2048       # sequence length
D = 2048         # hidden
H = 16           # heads
HD = D // H      # head dim = 128
W = 8            # cores
HPC = H // W     # heads per core = 2
DPC = HPC * HD   # features per core = 256
RPC = B * SEQ // W   # rows per core after re-shard = 512
FC = D // P      # feature chunks = 16
RT = B * SEQ     # total rows = 4096
KRC = SEQ // P   # key-row chunks per batch = 16
QRC = SEQ // 512  # query chunks of 512 per batch = 4

f32 = mybir.dt.float32
bf16 = mybir.dt.bfloat16
BF = ml_dtypes.bfloat16

INV_SQRT_HD = 1.0 / float(np.sqrt(HD))
Act = mybir.ActivationFunctionType

_CACHED_NC = None


def build_nc():
    nc = bacc.Bacc("TRN2", target_bir_lowering=False, debug=False)

    xT = nc.dram_tensor("xT", [D, RT], bf16, kind="ExternalInput")
    # weight chunks packed host-side: [p, fc, dpc]
    wq = nc.dram_tensor("wq", [P, FC, DPC], bf16, kind="ExternalInput")
    wk = nc.dram_tensor("wk", [P, FC, DPC], bf16, kind="ExternalInput")
    wv = nc.dram_tensor("wv", [P, FC, DPC], bf16, kind="ExternalInput")
    # bk is dropped entirely: softmax is invariant to a per-query-row
    # constant, and (q+bq).bk shifts every score in a row equally
    bq = nc.dram_tensor("bq", [DPC], f32, kind="ExternalInput")
    # wo packed host-side as 64 [128,512] tiles: t = (jc, hh, i)
    wo = nc.dram_tensor("wo", [P, 64, 512], bf16, kind="ExternalInput")
    # out rows: [0:256] = this core's batch-0 rows, [256:512] = batch-1 rows
    out = nc.dram_tensor("out", [RPC, D], bf16, kind="ExternalOutput")

    HB = RPC // B  # rows per core per batch = 256

    with tile.TileContext(nc) as tc:
        with (
            tc.tile_pool(name="persist", bufs=1) as persist,
            tc.tile_pool(name="dram", bufs=1, space="DRAM") as dram,
        ):
            # ---- persistent SBUF state ----
            qT_sb = persist.tile([P, HPC, RT], bf16)      # [hd, h, row]
            kT_sb = persist.tile([P, HPC, RT], bf16)
            v_sb = persist.tile([P, RT // P, DPC], bf16)  # [row%128, rowchunk, d]
            wo_sb = persist.tile([P, 64, 512], bf16)
            bq_sb = persist.tile([P, HPC], f32)
            ones_sb = persist.tile([P, 512], bf16)

            # one A2A per (head, batch): shard j = ctx.T for batch-b rows
            # [HB*j, HB*(j+1)) in head h's feature block
            a2a_in = [[dram.tile([W, HD, HB], bf16, name=f"a2a_in{h}{b}")
                       for b in range(B)] for h in range(HPC)]
            a2a_out = [[dram.tile([W, HD, HB], bf16, name=f"a2a_out{h}{b}")
                        for b in range(B)] for h in range(HPC)]
            cs_bounce = dram.tile([HPC * B * QRC // 2, 1024], f32,
                                  name="cs_bounce")

            # ones via memset: no DMA wait before the warmup matmuls
            nc.vector.memset(ones_sb[:], 1.0)

            # ---- HAM warmup: ~3.4us of small matmuls so the PE clock gate
            # opens just as the real work arrives ----
            with tc.tile_pool(name="warm_ps", bufs=1, space="PSUM") as warm_ps:
                wtile = warm_ps.tile([1, 256], f32, name="warm")
                for i in range(16):
                    nc.tensor.matmul(wtile[:], ones_sb[:, 0:1],
                                     ones_sb[:, 0:256], start=True, stop=True)

            # ---- phase 1: QKV projections ----
            with (
                tc.tile_pool(name="wproj", bufs=1) as wproj,
                tc.tile_pool(name="xtp", bufs=24) as xtp,
                tc.tile_pool(name="proj_ps", bufs=1, space="PSUM") as proj_ps,
            ):
                wq_sb = wproj.tile([P, FC, DPC], bf16)
                wk_sb = wproj.tile([P, FC, DPC], bf16)
                wv_sb = wproj.tile([P, FC, DPC], bf16)
                # all 3MB of qkv weights are consumed within rc0: round-robin
                # 0.25MB slices across the three DMA queues so no single
                # ring serializes them
                QROT = [nc.sync, nc.scalar, nc.gpsimd]
                w_i = [1]  # start at scalar so wq[0:2] leads that ring

                def stage_w(lo, hi):
                    for wsb, wdr in ((wq_sb, wq), (wk_sb, wk), (wv_sb, wv)):
                        QROT[w_i[0] % 3].dma_start(wsb[:, lo:hi, :],
                                                   wdr.ap()[:, lo:hi, :])
                        w_i[0] += 1

                stage_w(0, 2)          # scalar, gpsimd, sync
                # pre-issue all of rc0 and half of rc1 (24 tiles = the full
                # pool) so the DMA rings are deep before the PE catches up
                xt_pre = {}
                for j in range(24):
                    rc0, fc0 = j // FC, j % FC
                    xt = xtp.tile([P, 512], bf16, tag="xt",
                                  name=f"xt_pre{j}")
                    xq = QROT[fc0 % 3]
                    xq.dma_start(xt[:],
                                 xT.ap()[fc0 * P:(fc0 + 1) * P,
                                         rc0 * 512:(rc0 + 1) * 512])
                    xt_pre[(rc0, fc0)] = xt
                    if j == 5:
                        stage_w(2, 4)
                    if j == 11:
                        stage_w(4, 6)
                nc.gpsimd.dma_start(bq_sb[:],
                                    bq.ap().rearrange("(h p) -> p h", p=P))
                for rc in range(RT // 512):  # 8 row chunks of 512
                    q_ps = [proj_ps.tile([P, 512], f32, tag=f"q{i}",
                                         name=f"q_ps{i}")
                            for i in range(HPC)]
                    k_ps = [proj_ps.tile([P, 512], f32, tag=f"k{i}",
                                         name=f"k_ps{i}")
                            for i in range(HPC)]
                    v_ps = [proj_ps.tile([P, DPC], f32, tag=f"v{i}",
                                         name=f"v_ps{i}")
                            for i in range(4)]
                    for fc in range(FC):
                        if rc == 0 and fc in (2, 4, 6, 8, 10):
                            stage_w(fc + 4, fc + 6)
                        if (rc, fc) in xt_pre:
                            xt = xt_pre[(rc, fc)]
                        else:
                            xt = xtp.tile([P, 512], bf16, tag="xt")
                            xq = QROT[fc % 3]
                            xq.dma_start(
                                xt[:],
                                xT.ap()[fc * P:(fc + 1) * P,
                                        rc * 512:(rc + 1) * 512])
                        st = fc == 0
                        sp = fc == FC - 1
                        # interleave short-stream V matmuls between long
                        # Q/K streams so each V LDWEIGHTS hides behind a
                        # 512-cycle stream
                        for i in range(HPC):
                            nc.tensor.matmul(
                                q_ps[i][:],
                                wq_sb[:, fc, i * HD:(i + 1) * HD],
                                xt[:], start=st, stop=sp)
                            nc.tensor.matmul(
                                v_ps[2 * i][:],
                                xt[:, 2 * i * P:(2 * i + 1) * P],
                                wv_sb[:, fc, :], start=st, stop=sp)
                            nc.tensor.matmul(
                                k_ps[i][:],
                                wk_sb[:, fc, i * HD:(i + 1) * HD],
                                xt[:], start=st, stop=sp)
                            nc.tensor.matmul(
                                v_ps[2 * i + 1][:],
                                xt[:, (2 * i + 1) * P:(2 * i + 2) * P],
                                wv_sb[:, fc, :], start=st, stop=sp)
                    # PSUM -> SBUF; Q/K on ACT (with bias), V on DVE
                    for i in range(HPC):
                        nc.scalar.activation(
                            qT_sb[:, i, rc * 512:(rc + 1) * 512],
                            q_ps[i][:], Act.Identity,
                            bias=bq_sb[:, i:i + 1])
                        nc.vector.tensor_copy(
                            kT_sb[:, i, rc * 512:(rc + 1) * 512],
                            k_ps[i][:])
                    for s4 in range(4):
                        nc.vector.tensor_copy(
                            v_sb[:, rc * 4 + s4, :], v_ps[s4][:])

            # ctxl staging tiles (consumed by phase 3)
            ctxl_pool = tc.tile_pool(name="ctxl", bufs=1)
            ctxlp = ctxl_pool.__enter__()
            ctxl = [[ctxlp.tile([P, W, HB], bf16, name=f"ctxl{h}{b}")
                     for b in range(B)] for h in range(HPC)]

            # ---- phase 2: attention; b-outer so batch-0 re-shards early ----
            # The kc pipeline is FLAT across qp and quarter boundaries: the
            # PV pair always trails the score pair by two steps globally, so
            # the next qp's scores fill what would otherwise be an exp-drain
            # wait at every qp end. ctx accumulators are double-buffered and
            # normalized DIRECTLY from PSUM (no staging copy).
            with (
                tc.tile_pool(name="attn_sb", bufs=5) as attn_sb,
                tc.tile_pool(name="acc_sb", bufs=2) as acc_sbp,
                tc.tile_pool(name="norm_sb", bufs=3) as norm_sb,
                tc.tile_pool(name="st_ps", bufs=2, space="PSUM") as st_psp,
                tc.tile_pool(name="acc_ps", bufs=2, space="PSUM") as acc_psp,
            ):
                # deferred work carried across qp iterations:
                #   pend_cs  -> colsum MMs + reciprocal + broadcast (kc2)
                #   pend_out -> normalize from PSUM + ship shards (kc8)
                pend_cs = None
                pend_out = None
                pvq = []  # pending PV closures, popped two steps behind

                def flush_cs(fast=False):
                    nonlocal pend_cs, pend_out
                    if pend_cs is None:
                        return
                    ch, cb, cqp, acc2, ctx2, last = pend_cs
                    cs2 = st_psp.tile([P, 1024], f32, tag="st", name="cs2")
                    nc.tensor.matmul(cs2[0:1, 0:512], ones_sb[:, 0:1],
                                     acc2[:, 0:512], start=True, stop=True)
                    nc.tensor.matmul(cs2[0:1, 512:1024], ones_sb[:, 0:1],
                                     acc2[:, 512:1024], start=True, stop=True)
                    rcp = norm_sb.tile([1, 1024], f32, tag="rcp")
                    nc.vector.reciprocal_approx_fast(rcp[:], cs2[0:1, :])
                    src_ctx = ctx2
                    if fast:
                        # final qp: phase 3 reuses these PSUM banks, so evict
                        # ctx to SBUF immediately instead of holding the
                        # banks hostage to the broadcast round-trip; the
                        # bounce rides sync (idle here), clear of the qp0
                        # shard writes still draining on gpsimd
                        ctxu = norm_sb.tile([P, 1024], f32, tag="ctxu")
                        nc.vector.tensor_copy(ctxu[:], ctx2[:])
                        src_ctx = ctxu
                    bq_ = nc.sync if fast else nc.gpsimd
                    # partition-broadcast needs a DRAM bounce; the legs ride
                    # a queue nothing can head-of-line block
                    slot = (ch * B + cb) * (QRC // 2) + cqp
                    bq_.dma_start(cs_bounce[slot:slot + 1, :], rcp[:])
                    bc = norm_sb.tile([P, 1024], f32, tag="bc")
                    bq_.dma_start(
                        bc[:],
                        cs_bounce[slot:slot + 1, :].to_broadcast([P, 1024]))
                    pend_out = (ch, cb, cqp, src_ctx, bc, last)
                    pend_cs = None

                def flush_out():
                    nonlocal pend_out
                    if pend_out is None:
                        return
                    fh, fb, fqp, fctx2, fbc, last = pend_out
                    ctxn = norm_sb.tile([P, 1024], bf16, tag="ctxn")
                    nc.vector.tensor_mul(ctxn[:], fctx2[:], fbc[:])
                    for s4 in range(4):
                        nc.gpsimd.dma_start(
                            a2a_in[fh][fb][4 * fqp + s4, :, :],
                            ctxn[:, s4 * HB:(s4 + 1) * HB])
                    if last:
                        nc.gpsimd.collective_compute(
                            "AllToAll", mybir.AluOpType.bypass,
                            replica_groups=[list(range(W))],
                            ins=[a2a_in[fh][fb][:]], outs=[a2a_out[fh][fb][:]])
                    pend_out = None

                quarter = 0
                for b in range(B):
                    for h in range(HPC):
                        if b == 1:
                            # batch-0 ctxl loads, one head per quarter, on
                            # the sync queue: they may WAIT on the matching
                            # A2A, and nothing in the attention pipeline may
                            # queue behind that wait
                            for i in range(W):
                                nc.sync.dma_start(
                                    ctxl[h][0][:, i, :],
                                    a2a_out[h][0][i, :, :])
                        for qp in range(QRC // 2):
                            ctx2 = acc_psp.tile([P, 1024], f32, tag="ctx",
                                                name="ctx2")
                            acc2 = acc_sbp.tile([P, 1024], bf16, tag="acc",
                                                name="acc2")
                            q0 = b * SEQ + qp * 1024
                            pts = {}

                            def mk_pv(kc, ctx2=ctx2, pts=pts, b=b, h=h):
                                def pv():
                                    vc = v_sb[:, b * KRC + kc,
                                              h * HD:(h + 1) * HD]
                                    nc.tensor.matmul(
                                        ctx2[:, 0:512], vc,
                                        pts[kc][:, 0:512],
                                        start=kc == 0, stop=kc == KRC - 1)
                                    nc.tensor.matmul(
                                        ctx2[:, 512:1024], vc,
                                        pts[kc][:, 512:1024],
                                        start=kc == 0, stop=kc == KRC - 1)
                                return pv

                            for kc in range(KRC):
                                # Wo streams in during the first two
                                # quarters, on the sync queue only
                                if quarter < 2 and kc % 4 == 2:
                                    c = (quarter * 2 + qp) * 4 + kc // 4
                                    nc.sync.dma_start(
                                        wo_sb[:, c * 4:(c + 1) * 4, :],
                                        wo.ap()[:, c * 4:(c + 1) * 4, :])
                                st2 = st_psp.tile([P, 1024], f32, tag="st",
                                                  name="st2")
                                kT_c = kT_sb[:, h, b * SEQ + kc * P:
                                             b * SEQ + (kc + 1) * P]
                                nc.tensor.matmul(
                                    st2[:, 0:512], kT_c,
                                    qT_sb[:, h, q0:q0 + 512],
                                    start=True, stop=True)
                                nc.tensor.matmul(
                                    st2[:, 512:1024], kT_c,
                                    qT_sb[:, h, q0 + 512:q0 + 1024],
                                    start=True, stop=True)
                                if len(pvq) >= 2:
                                    pvq.pop(0)()   # PV from two steps back
                                if kc == 2:
                                    flush_cs()     # prev qp's colsum+rcp
                                if kc == 8:
                                    flush_out()    # prev qp's normalize+ship
                                pt2 = attn_sb.tile([P, 1024], bf16, tag="pt",
                                                   name="pt2")
                                # ONE exp per kc: the qc pair shares the
                                # instruction so the PSUM-access bubble is
                                # paid once, keeping ACT under the PE cadence
                                nc.scalar.activation(
                                    pt2[:], st2[:], Act.Exp,
                                    scale=INV_SQRT_HD)
                                if kc == 0:
                                    nc.vector.tensor_copy(acc2[:], pt2[:])
                                else:
                                    nc.vector.tensor_add(acc2[:], acc2[:],
                                                         pt2[:])
                                pts[kc] = pt2
                                pvq.append(mk_pv(kc))
                            pend_cs = (h, b, qp, acc2, ctx2,
                                       qp == QRC // 2 - 1)
                        quarter += 1
                # epilogue: launch the final colsum/reciprocal bounce
                # between the two drain PVs so its round-trip overlaps the
                # drain (phase 3 waits on these ctx banks)
                pvq.pop(0)()
                pvq.pop(0)()
                flush_cs(fast=True)
                flush_out()

            # ---- phase 3: output projection ----
            # out rows [0:256] come from b=0 shards, [256:512] from b=1
            # one (jc,r2) bank accumulates 16 back-to-back matmuls, then
            # evicts (cast+DMA) while the next bank accumulates
            # two passes per batch: the hh=0 sweep over all 8 banks consumes
            # only that head's A2A, so a late second-head collective can
            # never stall a bank mid-accumulation
            with (
                tc.tile_pool(name="osb", bufs=4) as osbp,
                tc.tile_pool(name="o_ps", bufs=1, space="PSUM") as o_psp,
            ):
                # batch-1 ctxl loads ride scalar: the scheduler hoists
                # A2A-gated DMAs to just after a queue's last attention work,
                # so they must sit where head-of-line blocking is free --
                # scalar is idle after the last exp and carries nothing else
                # (out tiles go on sync, the normalize chain on gpsimd)
                for hh in range(HPC):
                    for i in range(W):
                        nc.scalar.dma_start(ctxl[hh][1][:, i, :],
                                            a2a_out[hh][1][i, :, :])
                t = 0
                for bb in range(B):
                    o_ps = {}
                    for hh in range(HPC):
                        for r2 in range(HB // P):
                            for jc in range(D // 512):
                                if hh == 0:
                                    o_ps[(r2, jc)] = o_psp.tile(
                                        [P, 512], f32, tag=f"o{r2}{jc}",
                                        name=f"o_ps{r2}{jc}")
                                for i in range(W):
                                    st = hh == 0 and i == 0
                                    sp = hh == HPC - 1 and i == W - 1
                                    nc.tensor.matmul(
                                        o_ps[(r2, jc)][:],
                                        ctxl[hh][bb][:, i,
                                                     r2 * P:(r2 + 1) * P],
                                        wo_sb[:, jc * 16 + hh * 8 + i, :],
                                        start=st, stop=sp)
                                if hh == HPC - 1:
                                    o_sb = osbp.tile([P, 512], bf16,
                                                     tag="osb")
                                    nc.vector.tensor_copy(
                                        o_sb[:], o_ps[(r2, jc)][:])
                                    nc.sync.dma_start(
                                        out.ap()[(bb * 2 + r2) * P:
                                                 (bb * 2 + r2 + 1) * P,
                                                 jc * 512:(jc + 1) * 512],
                                        o_sb[:])
                                    t += 1
            ctxl_pool.__exit__(None, None, None)

    nc.compile()
    return nc


def kernel(x, Wq, bq, Wk, bk, Wv, bv, Wo, bo, _run_kwargs=None):
    global _CACHED_NC
    if _CACHED_NC is None:
        _CACHED_NC = build_nc()
    nc = _CACHED_NC

    x = np.asarray(x, dtype=np.float32)
    Wq = np.asarray(Wq, dtype=np.float32)
    Wk = np.asarray(Wk, dtype=np.float32)
    Wv = np.asarray(Wv, dtype=np.float32)
    Wo = np.asarray(Wo, dtype=np.float32)
    bq = np.asarray(bq, dtype=np.float32)
    bk = np.asarray(bk, dtype=np.float32)
    bv = np.asarray(bv, dtype=np.float32)
    bo = np.asarray(bo, dtype=np.float32)

    xT = np.ascontiguousarray(x.reshape(RT, D).T).astype(BF)   # [D, RT]
    bo_eff = (bo + Wo @ bv).astype(np.float32)                 # [D]

    def pack_w(Wslice):  # [DPC, D] -> [P, FC, DPC] (chunks on free axis)
        wT = Wslice.T.astype(BF)                # [D, DPC]
        return np.ascontiguousarray(
            wT.reshape(FC, P, DPC).transpose(1, 0, 2))

    # wo tiles t=(jc,hh,i): woT[i*DPC+hh*HD :+128, jc*512 :+512]
    woT = Wo.T.astype(BF)                       # [D, D]
    wo_p = np.empty((P, 64, 512), dtype=BF)
    for jc in range(4):
        for hh in range(HPC):
            for i in range(W):
                t = jc * 16 + hh * 8 + i
                r0 = i * DPC + hh * HD
                wo_p[:, t, :] = woT[r0:r0 + P, jc * 512:(jc + 1) * 512]
    wo_p = np.ascontiguousarray(wo_p)

    in_maps = []
    for i in range(W):
        sl = slice(i * DPC, (i + 1) * DPC)
        in_maps.append({
            "xT": xT,
            "wq": pack_w(Wq[sl, :]),
            "wk": pack_w(Wk[sl, :]),
            "wv": pack_w(Wv[sl, :]),
            "bq": np.ascontiguousarray(bq[sl]),
            "wo": wo_p,
        })

    kw = _run_kwargs or {}
    res = run_bass_kernel_spmd(nc, in_maps, core_ids=list(range(W)), **kw)

    HB = RPC // B
    full = np.empty((RT, D), dtype=np.float32)
    for i in range(W):
        o = res.results[i]["out"].astype(np.float32)
        full[i * HB:(i + 1) * HB, :] = o[:HB]              # batch 0 rows
        full[SEQ + i * HB:SEQ + (i + 1) * HB, :] = o[HB:]  # batch 1 rows
    full += bo_eff[None, :]
    out = full.reshape(B, SEQ, D)
    if kw:
        kernel.last_results = res
    return out
